# revision 1
# baseline (speedup 1.0000x reference)
"""DeepLSTM (B=32, T=512, I=256, H=512, L=4) Trainium2 kernel, v2.

Data-parallel over batch (8 cores x B_LOC=4). Per core:
- All weights resident in SBUF as bf16 (128KB/partition), loaded once.
- Diagonal layer wavefront: round r runs layer l on chunk (r-l) for all 4
  layers as independent streams, interleaved step-by-step so the per-step
  serial chain (matmul -> act -> vector -> transpose) of each stream hides
  behind the other streams' engine work. Warm-up/drain rounds compute zeros
  (zero input + zero bias -> h,c stay exactly 0), so no masking is needed.
- bf16 matmuls: 1 cycle/moving-column on the PE (fp32 is 4).
- Hidden state chunk handoff between layers via double-buffered (round
  parity) SBUF history buffers; h^T produced by one PE transpose per step.
"""
import sys

if '/opt/trn_rl_repo' not in sys.path:
    sys.path.insert(0, '/opt/trn_rl_repo')

import numpy as np

B, T, I, H, L = 32, 512, 256, 512, 4
N_CORES = 8
B_LOC = B // N_CORES
C = 8            # timesteps per chunk (wavefront granularity)
NCH = T // C     # real chunks
R2 = ((NCH + L - 1) + 1) // 2 * 2   # rounds, padded to even
NSLOT = R2 + 2   # x slots incl. prefetch lookahead
NS = 4           # strips (H/128)
G = 4            # gates
KC = 4           # k-chunks of 128
CB = C * B_LOC   # DMA index stride per round


def _bf16():
    import concourse.mybir as mybir
    return mybir.dt.np(mybir.dt.bfloat16)


def _pack_w(wlist_g):
    K = wlist_g[0].shape[0]
    W4 = np.stack(wlist_g, axis=0).astype(np.float32)
    if K < H:
        W4 = np.concatenate([W4, np.zeros((G, H - K, H), np.float32)], axis=1)
    W5 = W4.reshape(G, KC, 128, NS, 128)
    return np.ascontiguousarray(W5.transpose(1, 2, 3, 0, 4))


def _pack_xT(x_shard):
    B_l, T_, I_ = x_shard.shape
    xp = np.zeros((B_l, NSLOT * C, H), np.float32)
    xp[:, :T_, :I_] = x_shard
    xt = xp.reshape(B_l, NSLOT * C, KC, 128).transpose(2, 3, 1, 0)
    return np.ascontiguousarray(xt.reshape(KC, 128, NSLOT * C * B_l))


_NC_CACHE = {}


def _build_nc():
    if 'nc' in _NC_CACHE:
        return _NC_CACHE['nc']
    import concourse.bacc as bacc
    import concourse.tile as tile
    import concourse.mybir as mybir
    from concourse.bass import ds
    from concourse.masks import make_identity

    f32 = mybir.dt.float32
    bf16 = mybir.dt.bfloat16
    AF = mybir.ActivationFunctionType

    nc = bacc.Bacc("TRN2", target_bir_lowering=False, debug=False)
    w_dram = nc.dram_tensor("w", [L, 2, KC, 128, NS, G, 128], bf16,
                            kind="ExternalInput")
    xt_dram = nc.dram_tensor("xt", [KC, 128, NSLOT * CB], bf16,
                             kind="ExternalInput")
    bsel_dram = nc.dram_tensor("bsel", [128, 128], bf16, kind="ExternalInput")
    bias_dram = nc.dram_tensor("biasp", [L, 128, G, 128], bf16,
                               kind="ExternalInput")
    out_dram = nc.dram_tensor("ht_out", [KC, 128, R2 * CB], bf16,
                              kind="ExternalOutput")

    with tile.TileContext(nc) as tc:
        with tc.tile_pool(name="persist", bufs=1) as pp, \
             tc.tile_pool(name="work", bufs=3) as wk, \
             tc.tile_pool(name="psg", bufs=6, space="PSUM") as psg, \
             tc.tile_pool(name="pst", bufs=2, space="PSUM") as pst:

            # f32 scratch for init (memset/iota on bf16 tiles trips walrus
            # ISA checks; build in f32 and cast-copy)
            identf = pp.tile([128, 128], f32)
            make_identity(nc, identf[:])
            ident = pp.tile([128, 128], bf16)
            nc.vector.tensor_copy(ident[:], identf[:])
            zerof = pp.tile([128, KC * (C + 1) * B_LOC], f32)
            nc.gpsimd.memset(zerof[:], 0.0)
            bsel = pp.tile([128, 128], bf16)
            nc.sync.dma_start(out=bsel[:], in_=bsel_dram[:])
            biasp = pp.tile([128, L, G, 128], bf16)
            nc.sync.dma_start(out=biasp[:],
                              in_=bias_dram.rearrange("l p g u -> p l g u"))
            zcol = pp.tile([128, 1], bf16)
            nc.vector.tensor_copy(zcol[:], zerof[:, 0:1])

            w_sb = pp.tile([128, L, 2, KC, NS, G, 128], bf16, name="wres")
            for l in range(L):
                for s in range(2):
                    for k in range(KC):
                        nc.sync.dma_start(out=w_sb[:, l, s, k],
                                          in_=w_dram[l, s, k])

            hist = [[pp.tile([128, KC, C + 1, B_LOC], bf16,
                             name=f"hist{l}_{p}") for p in range(2)]
                    for l in range(L)]
            xbuf = [pp.tile([128, KC, C, B_LOC], bf16, name=f"xb{p}")
                    for p in range(2)]
            c_state = [pp.tile([128, 128], f32, name=f"cst{l}")
                       for l in range(L)]
            for l in range(L):
                for p in range(2):
                    nc.vector.tensor_copy(
                        hist[l][p][:].rearrange("p k t b -> p (k t b)"),
                        zerof[:])
                nc.gpsimd.memset(c_state[l][:], 0.0)

            # prologue: chunk 0 into xbuf[0], chunk 1 into xbuf[1]
            for p in range(2):
                nc.sync.dma_start(
                    out=xbuf[p][:].rearrange("p k t b -> p k (t b)"),
                    in_=xt_dram.rearrange("k p n -> p k n")[:, :,
                                                           p * CB:(p + 1) * CB])

            def step(l, t, p):
                gates = psg.tile([128, G, 128], f32, tag="g")
                nc.tensor.matmul(gates[:, :, :], bsel[:], biasp[:, l, :, :],
                                 start=True, stop=False)
                for s in range(2):
                    for k in range(KC):
                        if s == 0:
                            stat = hist[l][p][:, k, t, :]
                        elif l == 0:
                            stat = xbuf[p][:, k, t, :]
                        else:
                            stat = hist[l - 1][1 - p][:, k, t + 1, :]
                        for j in range(NS):
                            nc.tensor.matmul(
                                gates[32 * j:32 * j + B_LOC, :, :],
                                stat,
                                w_sb[:, l, s, k, j, :, :],
                                start=False, stop=False,
                                tile_position=(0, 32 * j),
                            )
                nc.tensor.matmul(gates[:, 0, 0:1], bsel[:], zcol[:],
                                 start=False, stop=True)
                gs = wk.tile([128, G, 128], f32, tag=f"gs{l}")
                nc.scalar.activation(gs[:, 0:3, :], gates[:, 0:3, :],
                                     AF.Sigmoid)
                nc.scalar.activation(gs[:, 3, :], gates[:, 3, :], AF.Tanh)
                fc = wk.tile([128, 128], f32, tag=f"fc{l}")
                ic = wk.tile([128, 128], f32, tag=f"ic{l}")
                nc.vector.tensor_mul(fc[:], gs[:, 1, :], c_state[l][:])
                nc.vector.tensor_mul(ic[:], gs[:, 0, :], gs[:, 3, :])
                nc.vector.tensor_add(c_state[l][:], fc[:], ic[:])
                th = wk.tile([128, 128], f32, tag=f"th{l}")
                nc.scalar.activation(th[:], c_state[l][:], AF.Tanh)
                h_sb = wk.tile([128, 128], bf16, tag=f"h{l}")
                nc.vector.tensor_mul(h_sb[:], gs[:, 2, :], th[:])
                tp = pst.tile([128, KC, 32], bf16, tag="tp")
                nc.tensor.transpose(
                    tp[:, :, :].rearrange("p k b -> p (k b)"),
                    h_sb[:], ident[:])
                nc.vector.tensor_copy(hist[l][p][:, :, t + 1, :],
                                      tp[:, :, 0:B_LOC])

            with tc.For_i(0, R2 * CB, 2 * CB) as iv:
                for p in range(2):
                    roff = (iv + p * CB) if p else iv
                    for t in range(C):
                        for l in range(L):
                            step(l, t, p)
                    # prefetch x chunk r+2 into the buffer just consumed
                    # (must be emitted AFTER this round's reads of xbuf[p])
                    nc.sync.dma_start(
                        out=xbuf[p][:].rearrange("p k t b -> p k (t b)"),
                        in_=xt_dram.rearrange("k p n -> p k n")
                            [:, :, ds(roff + 2 * CB, CB)],
                    )
                    for l in range(L):
                        nc.vector.tensor_copy(hist[l][1 - p][:, :, 0, :],
                                              hist[l][p][:, :, C, :])
                    nc.sync.dma_start(
                        out=out_dram.rearrange("k p n -> p k n")
                            [:, :, ds(roff, CB)],
                        in_=hist[L - 1][p][:, :, 1:C + 1, :].rearrange(
                            "p k t b -> p k (t b)"),
                    )
    nc.compile()
    _NC_CACHE['nc'] = nc
    return nc


def kernel(inputs, Wxi0, Wxf0, Wxo0, Wxc0, Wxi, Wxf, Wxo, Wxc,
           Whi, Whf, Who, Whc, bi, bf, bo, bc, _trace=False):
    from concourse.bass_utils import run_bass_kernel_spmd

    bft = _bf16()
    inputs = np.asarray(inputs, dtype=np.float32)
    Wx_l = [[np.asarray(Wxi0), np.asarray(Wxf0), np.asarray(Wxo0),
             np.asarray(Wxc0)]]
    for li in range(L - 1):
        Wx_l.append([np.asarray(Wxi)[li], np.asarray(Wxf)[li],
                     np.asarray(Wxo)[li], np.asarray(Wxc)[li]])
    Wh_l = [[np.asarray(Whi)[li], np.asarray(Whf)[li], np.asarray(Who)[li],
             np.asarray(Whc)[li]] for li in range(L)]
    b_l = [[np.asarray(bi)[li], np.asarray(bf)[li], np.asarray(bo)[li],
            np.asarray(bc)[li]] for li in range(L)]

    wpk = np.zeros((L, 2, KC, 128, NS, G, 128), np.float32)
    for l in range(L):
        wpk[l, 0] = _pack_w(Wh_l[l])
        wpk[l, 1] = _pack_w(Wx_l[l])
    wpk = wpk.astype(bft)

    bsel = np.zeros((128, 128), np.float32)
    for j in range(NS):
        bsel[j, 32 * j:32 * (j + 1)] = 1.0
    bsel = bsel.astype(bft)
    biasp = np.zeros((L, 128, G, 128), np.float32)
    for l in range(L):
        for g in range(G):
            biasp[l, :NS, g, :] = b_l[l][g].reshape(NS, 128)
    biasp = biasp.astype(bft)

    nc = _build_nc()
    in_maps = []
    for cid in range(N_CORES):
        shard = inputs[cid * B_LOC:(cid + 1) * B_LOC]
        in_maps.append({
            "w": wpk,
            "xt": _pack_xT(shard).astype(bft),
            "bsel": bsel,
            "biasp": biasp,
        })
    res = run_bass_kernel_spmd(nc, in_maps, core_ids=list(range(N_CORES)),
                               trace=_trace)
    out = np.zeros((B, T, H), np.float32)
    for cid in range(N_CORES):
        ht = np.asarray(res.results[cid]["ht_out"]).astype(np.float32)
        ht = ht.reshape(KC, 128, R2 * C, B_LOC)
        ht = ht[:, :, (L - 1) * C:(L - 1) * C + T, :]
        out[cid * B_LOC:(cid + 1) * B_LOC] = ht.transpose(3, 2, 0, 1).reshape(
            B_LOC, T, H)
    if _trace:
        _NC_CACHE['last_result'] = res
    return out

